# revision 4
# baseline (speedup 1.0000x reference)
"""DeepseekV2 MoE MLP (grouped SwiGLU MLP) on 8 Trainium2 NeuronCores.

Strategy: expert-parallel. 32 experts / 8 cores = 4 experts per core; the
token groups (pre-sorted, contiguous per expert) travel with their expert,
so the "all-to-all" is a host-side slice/concat. Each core streams its
4 experts' weights (cast to bf16 on host -> halves HBM traffic) and runs:

    G = X_e @ Wg_e ; U = X_e @ Wu_e          (tokens stationary, weights moving)
    I = silu(G) * U                           (ACT + DVE, fp32 psum -> bf16)
    I^T via PE transpose (11x [64,128]->[128,64])
    O = I @ Wd_e                              (I^T stationary, Wd moving)

PSUM budget: G[64,1408](3) + U[64,1408](3) + T[128,64](1) + O[64,512](1) = 8 banks.
"""

import numpy as np
import ml_dtypes

T, HS, IMZ, E = 2048, 2048, 1408, 32
NCORES = 8
E_LOC = E // NCORES          # experts per core
TOK = 64                     # token slots per expert
TOKC = E_LOC * TOK           # token slots per core
KT_HS = HS // 128            # 16 contraction tiles for gate/up
MT_IMZ = IMZ // 128          # 11 contraction tiles for down
GU_SLICES = [(0, 512), (512, 512), (1024, 384)]   # IMZ n-slices (psum bank aligned)
W_CHUNK = 4                  # k-tiles per weight DMA chunk

_BF16 = ml_dtypes.bfloat16
_CACHE = {}


def _build_bass():
    import concourse.tile as tile
    import concourse.mybir as mybir
    from concourse import bacc
    from concourse.masks import make_identity

    f32 = mybir.dt.float32
    bf16 = mybir.dt.bfloat16

    nc = bacc.Bacc("TRN2", target_bir_lowering=False)
    xt_d = nc.dram_tensor("xt", [HS, TOKC], bf16, kind="ExternalInput")
    wg_d = nc.dram_tensor("wg", [E_LOC, HS, IMZ], bf16, kind="ExternalInput")
    wu_d = nc.dram_tensor("wu", [E_LOC, HS, IMZ], bf16, kind="ExternalInput")
    wd_d = nc.dram_tensor("wd", [E_LOC, IMZ, HS], bf16, kind="ExternalInput")
    out_d = nc.dram_tensor("out", [TOKC, HS], f32, kind="ExternalOutput")

    with tile.TileContext(nc) as tc:
        with (
            tc.tile_pool(name="const", bufs=1) as const_pool,
            tc.tile_pool(name="wg", bufs=3) as wg_pool,
            tc.tile_pool(name="wu", bufs=3) as wu_pool,
            tc.tile_pool(name="wd", bufs=3) as wd_pool,
            tc.tile_pool(name="act", bufs=2) as act_pool,
            tc.tile_pool(name="outp", bufs=2) as out_pool,
            tc.tile_pool(name="pg", bufs=1, space="PSUM") as pg_pool,
            tc.tile_pool(name="pu", bufs=1, space="PSUM") as pu_pool,
            tc.tile_pool(name="pt", bufs=1, space="PSUM") as pt_pool,
            tc.tile_pool(name="po", bufs=1, space="PSUM") as po_pool,
        ):
            ident = const_pool.tile([TOK, TOK], bf16)
            make_identity(nc, ident)

            xt_sb = const_pool.tile([128, KT_HS, TOKC], bf16)
            nc.sync.dma_start(
                out=xt_sb, in_=xt_d.ap().rearrange("(k p) t -> p k t", p=128)
            )

            for j in range(E_LOC):
                # ---- weight streaming DMAs (bufs throttle the prefetch) ----
                wg_t, wu_t = [], []
                for c in range(KT_HS // W_CHUNK):
                    r0 = c * W_CHUNK * 128
                    gt = wg_pool.tile([128, W_CHUNK, IMZ], bf16, tag="wg")
                    nc.sync.dma_start(
                        out=gt,
                        in_=wg_d.ap()[j, r0 : r0 + W_CHUNK * 128, :].rearrange(
                            "(kk p) n -> p kk n", p=128
                        ),
                    )
                    wg_t.append(gt)
                    ut = wu_pool.tile([128, W_CHUNK, IMZ], bf16, tag="wu")
                    nc.sync.dma_start(
                        out=ut,
                        in_=wu_d.ap()[j, r0 : r0 + W_CHUNK * 128, :].rearrange(
                            "(kk p) n -> p kk n", p=128
                        ),
                    )
                    wu_t.append(ut)
                wd_t = []
                for c, sz in enumerate((4, 4, 3)):
                    r0 = c * 4 * 128
                    dt_ = wd_pool.tile([128, sz, HS], bf16, tag="wd")
                    nc.sync.dma_start(
                        out=dt_,
                        in_=wd_d.ap()[j, r0 : r0 + sz * 128, :].rearrange(
                            "(kk p) n -> p kk n", p=128
                        ),
                    )
                    wd_t.append(dt_)

                # ---- gate/up grouped GEMMs: tokens stationary ----
                pg = pg_pool.tile([TOK, IMZ], f32, tag="pg")
                pu = pu_pool.tile([TOK, IMZ], f32, tag="pu")
                for k in range(KT_HS):
                    c, kk = divmod(k, W_CHUNK)
                    lhsT = xt_sb[:, k, j * TOK : (j + 1) * TOK]
                    for n0, nsz in GU_SLICES:
                        nc.tensor.matmul(
                            pg[:, n0 : n0 + nsz],
                            lhsT,
                            wg_t[c][:, kk, n0 : n0 + nsz],
                            start=(k == 0),
                            stop=(k == KT_HS - 1),
                        )
                    for n0, nsz in GU_SLICES:
                        nc.tensor.matmul(
                            pu[:, n0 : n0 + nsz],
                            lhsT,
                            wu_t[c][:, kk, n0 : n0 + nsz],
                            start=(k == 0),
                            stop=(k == KT_HS - 1),
                        )

                # ---- SwiGLU: inter = silu(G) * U  (bf16 out) ----
                sg = act_pool.tile([TOK, IMZ], f32, tag="sg")
                nc.scalar.activation(sg, pg, mybir.ActivationFunctionType.Silu)
                inter = act_pool.tile([TOK, IMZ], bf16, tag="inter")
                nc.vector.tensor_mul(inter, sg, pu)

                # ---- transpose inter -> interT [IMZ, TOK] ----
                interT = act_pool.tile([128, MT_IMZ, TOK], bf16, tag="interT")
                for m in range(MT_IMZ):
                    pt = pt_pool.tile([128, TOK], bf16, tag="pt")
                    nc.tensor.transpose(pt, inter[:, m * 128 : (m + 1) * 128], ident)
                    nc.any.tensor_copy(out=interT[:, m, :], in_=pt)

                # ---- down projection: interT stationary, Wd moving ----
                osb = out_pool.tile([TOK, HS], f32, tag="osb")
                for q in range(HS // 512):
                    po = po_pool.tile([TOK, 512], f32, tag="po")
                    for m in range(MT_IMZ):
                        c, mm = divmod(m, 4)
                        nc.tensor.matmul(
                            po,
                            interT[:, m, :],
                            wd_t[c][:, mm, q * 512 : (q + 1) * 512],
                            start=(m == 0),
                            stop=(m == MT_IMZ - 1),
                        )
                    nc.any.tensor_copy(out=osb[:, q * 512 : (q + 1) * 512], in_=po)
                nc.sync.dma_start(
                    out=out_d.ap()[j * TOK : (j + 1) * TOK, :], in_=osb
                )
    nc.compile()
    return nc


def _get_bass():
    if "nc" not in _CACHE:
        _CACHE["nc"] = _build_bass()
    return _CACHE["nc"]


def _numpy_fallback(x, gk, uk, dk, gs):
    out = np.zeros((x.shape[0], dk.shape[2]), np.float32)
    off = 0
    for e in range(gs.shape[0]):
        g = int(gs[e])
        if g <= 0:
            continue
        xs = x[off : off + g].astype(np.float32)
        gg = xs @ gk[e]
        uu = xs @ uk[e]
        inter = (gg / (1.0 + np.exp(-gg))) * uu
        out[off : off + g] = inter @ dk[e]
        off += g
    return out


def run_sharded(hidden_states, gate_kernel, up_kernel, down_kernel, group_sizes,
                trace=False):
    """Shard -> run on 8 cores -> gather. Returns (out, BassKernelResults)."""
    from concourse.bass_utils import run_bass_kernel_spmd

    x = np.ascontiguousarray(hidden_states, dtype=np.float32)
    gs = np.asarray(group_sizes).astype(np.int64)
    offs = np.concatenate([[0], np.cumsum(gs)])

    nc = _get_bass()
    in_maps = []
    for c in range(NCORES):
        xt = np.zeros((TOKC, HS), dtype=np.float32)
        for jj in range(E_LOC):
            e = c * E_LOC + jj
            g = int(gs[e])
            xt[jj * TOK : jj * TOK + g] = x[offs[e] : offs[e] + g]
        in_maps.append(
            {
                "xt": np.ascontiguousarray(xt.T).astype(_BF16),
                "wg": np.ascontiguousarray(
                    gate_kernel[c * E_LOC : (c + 1) * E_LOC]
                ).astype(_BF16),
                "wu": np.ascontiguousarray(
                    up_kernel[c * E_LOC : (c + 1) * E_LOC]
                ).astype(_BF16),
                "wd": np.ascontiguousarray(
                    down_kernel[c * E_LOC : (c + 1) * E_LOC]
                ).astype(_BF16),
            }
        )

    res = run_bass_kernel_spmd(nc, in_maps, core_ids=list(range(NCORES)),
                               trace=trace)

    out = np.zeros((x.shape[0], HS), np.float32)
    for c in range(NCORES):
        o = res.results[c]["out"]
        for jj in range(E_LOC):
            e = c * E_LOC + jj
            g = int(gs[e])
            out[offs[e] : offs[e] + g] = o[jj * TOK : jj * TOK + g]
    return out, res


def kernel(hidden_states, gate_kernel, up_kernel, down_kernel, group_sizes):
    gs = np.asarray(group_sizes)
    ok = (
        gs.shape == (E,)
        and int(gs.max(initial=0)) <= TOK
        and int(gs.sum()) <= T
        and hidden_states.shape == (T, HS)
        and gate_kernel.shape == (E, HS, IMZ)
        and up_kernel.shape == (E, HS, IMZ)
        and down_kernel.shape == (E, IMZ, HS)
    )
    if not ok:
        return _numpy_fallback(
            np.asarray(hidden_states, np.float32),
            np.asarray(gate_kernel, np.float32),
            np.asarray(up_kernel, np.float32),
            np.asarray(down_kernel, np.float32),
            gs.astype(np.int64),
        )
    out, _ = run_sharded(
        hidden_states, gate_kernel, up_kernel, down_kernel, group_sizes
    )
    return out


# revision 16
# speedup vs baseline: 1.2269x; 1.2269x over previous
"""DeepseekV2 MoE MLP (grouped SwiGLU MLP) on 8 Trainium2 NeuronCores.

Strategy: expert-parallel. 32 experts / 8 cores = 4 experts per core; the
token groups (pre-sorted, contiguous per expert) travel with their expert,
so the "all-to-all" is a host-side slice/concat. Each core streams its
4 experts' weights (cast to bf16 on host -> halves HBM traffic) and runs:

    G = X_e @ Wg_e ; U = X_e @ Wu_e          (tokens stationary, weights moving)
    I = silu(G) * U                           (ACT + DVE, fp32 psum -> bf16)
    I^T via PE transpose (11x [64,128]->[128,64])
    O = I @ Wd_e                              (I^T stationary, Wd moving)

gate/up weights are host-concatenated into one [HS, 2*IMZ] tensor per expert
so each weight DMA moves ~2.9 MB. The down matmul accumulates m-outer (all
11 IMZ k-tiles per output half) so wd chunks release progressively and the
next expert's DMAs can start early.

PSUM budget: G[64,1408](3) + U[64,1408](3) + shared{T[128,64]/O[64,1024]}(2) = 8 banks.
"""

import numpy as np
import ml_dtypes

T, HS, IMZ, E = 2048, 2048, 1408, 32
NCORES = 8
E_LOC = E // NCORES          # experts per core
TOK = 64                     # token slots per expert
TOKC = E_LOC * TOK           # token slots per core
KT_HS = HS // 128            # 16 contraction tiles for gate/up
MT_IMZ = IMZ // 128          # 11 contraction tiles for down
GU_SLICES = [(0, 512), (512, 512), (1024, 384)]   # IMZ n-slices (psum bank aligned)
W_CHUNK = 4                  # k-tiles per weight DMA chunk
WD_CHUNKS = (2, 2, 2, 2, 2, 1)   # m-tiles per down-weight DMA chunk

_BF16 = ml_dtypes.bfloat16
_CACHE = {}

# tuning knobs (A/B-able from bench scripts)
CONFIG = dict(
    wgu_bufs=4,
    wd_chunks=WD_CHUNKS,
    wd_bufs=6,
    wd_tile=2,
    e0_splits=(1, 1, 2, 4, 4, 4),
    split_store=True,
    pair_gu=False,
)


def _build_bass(cfg=None):
    cfg = dict(CONFIG if cfg is None else cfg)
    import concourse.tile as tile
    import concourse.mybir as mybir
    from concourse import bacc
    from concourse.masks import make_identity

    f32 = mybir.dt.float32
    bf16 = mybir.dt.bfloat16

    nc = bacc.Bacc("TRN2", target_bir_lowering=False)
    xt_d = nc.dram_tensor("xt", [HS, TOKC], bf16, kind="ExternalInput")
    wgu_d = nc.dram_tensor("wgu", [E_LOC, HS, 2 * IMZ], bf16, kind="ExternalInput")
    wd_d = nc.dram_tensor("wd", [E_LOC, IMZ, HS], bf16, kind="ExternalInput")
    out_d = nc.dram_tensor("out", [TOKC, HS], f32, kind="ExternalOutput")

    with tile.TileContext(nc) as tc:
        with (
            tc.tile_pool(name="const", bufs=1) as const_pool,
            tc.tile_pool(name="wgu", bufs=cfg["wgu_bufs"]) as wgu_pool,
            tc.tile_pool(name="wd", bufs=cfg["wd_bufs"]) as wd_pool,
            tc.tile_pool(name="act", bufs=2) as act_pool,
            tc.tile_pool(name="outp", bufs=2) as out_pool,
            tc.tile_pool(name="pg", bufs=1, space="PSUM") as pg_pool,
            tc.tile_pool(name="pu", bufs=1, space="PSUM") as pu_pool,
            tc.tile_pool(name="dp", bufs=1, space="PSUM") as dp_pool,
        ):
            ident = const_pool.tile([TOK, TOK], bf16)
            make_identity(nc, ident)

            xt_sb = const_pool.tile([128, KT_HS, TOKC], bf16)
            nc.sync.dma_start(
                out=xt_sb, in_=xt_d.ap().rearrange("(k p) t -> p k t", p=128)
            )

            if cfg["pair_gu"]:
                _build_paired(nc, tc, cfg, mybir, make_identity,
                              const_pool, wgu_pool, wd_pool, act_pool, out_pool,
                              pg_pool, pu_pool, dp_pool,
                              ident, xt_sb, wgu_d, wd_d, out_d)
            for j in (range(E_LOC) if not cfg["pair_gu"] else ()):
                # ---- weight streaming DMAs (bufs throttle the prefetch) ----
                # expert 0 uses fine-grained leading chunks so the first
                # matmuls start ~15us earlier on a cold pipe
                splits = cfg["e0_splits"] if j == 0 else (4, 4, 4, 4)
                wgu_t = []  # (tile, k0, nk)
                r0 = 0
                for nk in splits:
                    gt = wgu_pool.tile([128, W_CHUNK, 2 * IMZ], bf16, tag="wgu")
                    nc.sync.dma_start(
                        out=gt[:, :nk, :],
                        in_=wgu_d.ap()[j, r0 : r0 + nk * 128, :].rearrange(
                            "(kk p) n -> p kk n", p=128
                        ),
                    )
                    wgu_t.append((gt, r0 // 128, nk))
                    r0 += nk * 128
                wd_t = {}  # m-tile -> (tile, local idx)
                r0 = 0
                for sz in cfg["wd_chunks"]:
                    dt_ = wd_pool.tile([128, cfg["wd_tile"], HS], bf16, tag="wd")
                    nc.sync.dma_start(
                        out=dt_[:, :sz, :],
                        in_=wd_d.ap()[j, r0 : r0 + sz * 128, :].rearrange(
                            "(kk p) n -> p kk n", p=128
                        ),
                    )
                    for kk in range(sz):
                        wd_t[r0 // 128 + kk] = (dt_, kk)
                    r0 += sz * 128

                # ---- gate/up grouped GEMMs: tokens stationary, weights moving ----
                k_to_chunk = {}
                for gt, k0, nk in wgu_t:
                    for kk in range(nk):
                        k_to_chunk[k0 + kk] = (gt, kk)
                pg = pg_pool.tile([TOK, IMZ], f32, tag="pg")
                pu = pu_pool.tile([TOK, IMZ], f32, tag="pu")
                for k in range(KT_HS):
                    gt, kk = k_to_chunk[k]
                    lhsT = xt_sb[:, k, j * TOK : (j + 1) * TOK]
                    for n0, nsz in GU_SLICES:
                        nc.tensor.matmul(
                            pg[:, n0 : n0 + nsz],
                            lhsT,
                            gt[:, kk, n0 : n0 + nsz],
                            start=(k == 0),
                            stop=(k == KT_HS - 1),
                        )
                    for n0, nsz in GU_SLICES:
                        nc.tensor.matmul(
                            pu[:, n0 : n0 + nsz],
                            lhsT,
                            gt[:, kk, IMZ + n0 : IMZ + n0 + nsz],
                            start=(k == 0),
                            stop=(k == KT_HS - 1),
                        )

                # ---- SwiGLU: inter = silu(G) * U  (bf16 out) ----
                sg = act_pool.tile([TOK, IMZ], f32, tag="sg")
                nc.scalar.activation(sg, pg, mybir.ActivationFunctionType.Silu)
                inter = act_pool.tile([TOK, IMZ], bf16, tag="inter")
                nc.vector.tensor_mul(inter, sg, pu)

                # ---- transpose inter -> interT [IMZ, TOK] (PE transpose) ----
                interT = act_pool.tile([128, MT_IMZ, TOK], bf16, tag="interT")
                for m in range(MT_IMZ):
                    pt = dp_pool.tile([128, TOK], bf16, tag="dp")
                    nc.tensor.transpose(pt, inter[:, m * 128 : (m + 1) * 128], ident)
                    nc.vector.tensor_copy(out=interT[:, m, :], in_=pt)

                # ---- down projection: interT stationary, Wd moving, m-outer ----
                osb = out_pool.tile([TOK, HS], f32, tag="osb")
                for qh in range(2):
                    po = dp_pool.tile([TOK, 1024], f32, tag="dp")
                    for m in range(MT_IMZ):
                        dt_, mm = wd_t[m]
                        for q2 in range(2):
                            n0 = qh * 1024 + q2 * 512
                            nc.tensor.matmul(
                                po[:, q2 * 512 : (q2 + 1) * 512],
                                interT[:, m, :],
                                dt_[:, mm, n0 : n0 + 512],
                                start=(m == 0),
                                stop=(m == MT_IMZ - 1),
                            )
                    nc.vector.tensor_copy(
                        out=osb[:, qh * 1024 : (qh + 1) * 1024], in_=po
                    )
                    # out-store rides the ACT HWDGE queue: on the SP queue its
                    # wait-for-copy would head-of-line block the weight stream.
                    if cfg["split_store"]:
                        nc.scalar.dma_start(
                            out=out_d.ap()[
                                j * TOK : (j + 1) * TOK, qh * 1024 : (qh + 1) * 1024
                            ],
                            in_=osb[:, qh * 1024 : (qh + 1) * 1024],
                        )
                if not cfg["split_store"]:
                    nc.scalar.dma_start(
                        out=out_d.ap()[j * TOK : (j + 1) * TOK, :], in_=osb
                    )
    nc.compile()
    return nc


def _build_paired(nc, tc, cfg, mybir, make_identity,
                  const_pool, wgu_pool, wd_pool, act_pool, out_pool,
                  pg_pool, pu_pool, dp_pool,
                  ident, xt_sb, wgu_d, wd_d, out_d):
    """Gate/up + down with expert pairs sharing the PE array via column tiling:
    expert A on PSUM partitions 0-63, expert B on 64-127. Concurrent matmuls
    on disjoint column groups roughly halve PE busy time."""
    f32 = mybir.dt.float32
    bf16 = mybir.dt.bfloat16
    ident128 = const_pool.tile([128, 128], bf16)
    make_identity(nc, ident128)

    for p in range(E_LOC // 2):
        jA, jB = 2 * p, 2 * p + 1
        # ---- weight DMAs, interleaved A/B ----
        splits = cfg["e0_splits"] if p == 0 else (4, 4, 4, 4)
        chunks = {jA: [], jB: []}
        r0s = {jA: 0, jB: 0}
        for nk in splits:
            for j in (jA, jB):
                r0 = r0s[j]
                gt = wgu_pool.tile([128, W_CHUNK, 2 * IMZ], bf16, tag="wgu")
                nc.sync.dma_start(
                    out=gt[:, :nk, :],
                    in_=wgu_d.ap()[j, r0 : r0 + nk * 128, :].rearrange(
                        "(kk p) n -> p kk n", p=128
                    ),
                )
                chunks[j].append((gt, r0 // 128, nk))
                r0s[j] = r0 + nk * 128
        wd_t = {jA: {}, jB: {}}
        for j in (jA, jB):
            r0 = 0
            for sz in cfg["wd_chunks"]:
                dt_ = wd_pool.tile([128, cfg["wd_tile"], HS], bf16, tag="wd")
                nc.sync.dma_start(
                    out=dt_[:, :sz, :],
                    in_=wd_d.ap()[j, r0 : r0 + sz * 128, :].rearrange(
                        "(kk p) n -> p kk n", p=128
                    ),
                )
                for kk in range(sz):
                    wd_t[j][r0 // 128 + kk] = (dt_, kk)
                r0 += sz * 128

        k_to_chunk = {jA: {}, jB: {}}
        for j in (jA, jB):
            for gt, k0, nk in chunks[j]:
                for kk in range(nk):
                    k_to_chunk[j][k0 + kk] = (gt, kk)

        # ---- paired gate/up: A -> psum rows 0-63, B -> rows 64-127 ----
        pg = pg_pool.tile([128, IMZ], f32, tag="pg")
        pu = pu_pool.tile([128, IMZ], f32, tag="pu")
        for k in range(KT_HS):
            for ps, base in ((pg, 0), (pu, IMZ)):
                for n0, nsz in GU_SLICES:
                    for j, row0 in ((jA, 0), (jB, 64)):
                        gt, kk = k_to_chunk[j][k]
                        nc.tensor.matmul(
                            ps[row0 : row0 + 64, n0 : n0 + nsz],
                            xt_sb[:, k, j * TOK : (j + 1) * TOK],
                            gt[:, kk, base + n0 : base + n0 + nsz],
                            start=(k == 0),
                            stop=(k == KT_HS - 1),
                        )

        # ---- SwiGLU for the whole pair ----
        sg = act_pool.tile([128, IMZ], f32, tag="sg")
        nc.scalar.activation(sg, pg, mybir.ActivationFunctionType.Silu)
        inter = act_pool.tile([128, IMZ], bf16, tag="inter")
        nc.vector.tensor_mul(inter, sg, pu)

        # ---- full 128x128 transposes: interT[:, m, 0:64]=A, [64:128]=B ----
        interT = act_pool.tile([128, MT_IMZ, 128], bf16, tag="interT")
        for m in range(MT_IMZ):
            pt = dp_pool.tile([128, 128], bf16, tag="dp")
            nc.tensor.transpose(pt, inter[:, m * 128 : (m + 1) * 128], ident128)
            nc.vector.tensor_copy(out=interT[:, m, :], in_=pt)

        # ---- down per expert (interT columns select the expert) ----
        for j, row0 in ((jA, 0), (jB, 64)):
            osb = out_pool.tile([TOK, HS], f32, tag="osb")
            for qh in range(2):
                po = dp_pool.tile([TOK, 1024], f32, tag="dp")
                for m in range(MT_IMZ):
                    dt_, mm = wd_t[j][m]
                    for q2 in range(2):
                        n0 = qh * 1024 + q2 * 512
                        nc.tensor.matmul(
                            po[:, q2 * 512 : (q2 + 1) * 512],
                            interT[:, m, row0 : row0 + 64],
                            dt_[:, mm, n0 : n0 + 512],
                            start=(m == 0),
                            stop=(m == MT_IMZ - 1),
                        )
                nc.vector.tensor_copy(
                    out=osb[:, qh * 1024 : (qh + 1) * 1024], in_=po
                )
                nc.scalar.dma_start(
                    out=out_d.ap()[
                        j * TOK : (j + 1) * TOK, qh * 1024 : (qh + 1) * 1024
                    ],
                    in_=osb[:, qh * 1024 : (qh + 1) * 1024],
                )


def _get_bass(cfg=None):
    key = tuple(sorted((k, str(v)) for k, v in (CONFIG if cfg is None else cfg).items()))
    if key not in _CACHE:
        _CACHE[key] = _build_bass(cfg)
    return _CACHE[key]


def _numpy_fallback(x, gk, uk, dk, gs):
    out = np.zeros((x.shape[0], dk.shape[2]), np.float32)
    off = 0
    for e in range(gs.shape[0]):
        g = int(gs[e])
        if g <= 0:
            continue
        xs = x[off : off + g].astype(np.float32)
        gg = xs @ gk[e]
        uu = xs @ uk[e]
        inter = (gg / (1.0 + np.exp(-gg))) * uu
        out[off : off + g] = inter @ dk[e]
        off += g
    return out


def run_sharded(hidden_states, gate_kernel, up_kernel, down_kernel, group_sizes,
                trace=False, cfg=None):
    """Shard -> run on 8 cores -> gather. Returns (out, BassKernelResults)."""
    from concourse.bass_utils import run_bass_kernel_spmd

    x = np.ascontiguousarray(hidden_states, dtype=np.float32)
    gs = np.asarray(group_sizes).astype(np.int64)
    offs = np.concatenate([[0], np.cumsum(gs)])

    nc = _get_bass(cfg)
    in_maps = []
    for c in range(NCORES):
        xt = np.zeros((TOKC, HS), dtype=np.float32)
        for jj in range(E_LOC):
            e = c * E_LOC + jj
            g = int(gs[e])
            xt[jj * TOK : jj * TOK + g] = x[offs[e] : offs[e] + g]
        sl = slice(c * E_LOC, (c + 1) * E_LOC)
        wgu = np.concatenate(
            [
                np.asarray(gate_kernel[sl], np.float32),
                np.asarray(up_kernel[sl], np.float32),
            ],
            axis=2,
        )
        in_maps.append(
            {
                "xt": np.ascontiguousarray(xt.T).astype(_BF16),
                "wgu": wgu.astype(_BF16),
                "wd": np.ascontiguousarray(down_kernel[sl]).astype(_BF16),
            }
        )

    res = run_bass_kernel_spmd(nc, in_maps, core_ids=list(range(NCORES)),
                               trace=trace)

    out = np.zeros((x.shape[0], HS), np.float32)
    for c in range(NCORES):
        o = res.results[c]["out"]
        for jj in range(E_LOC):
            e = c * E_LOC + jj
            g = int(gs[e])
            out[offs[e] : offs[e] + g] = o[jj * TOK : jj * TOK + g]
    return out, res


def kernel(hidden_states, gate_kernel, up_kernel, down_kernel, group_sizes):
    gs = np.asarray(group_sizes)
    ok = (
        gs.shape == (E,)
        and int(gs.max(initial=0)) <= TOK
        and int(gs.sum()) <= T
        and hidden_states.shape == (T, HS)
        and gate_kernel.shape == (E, HS, IMZ)
        and up_kernel.shape == (E, HS, IMZ)
        and down_kernel.shape == (E, IMZ, HS)
    )
    if not ok:
        return _numpy_fallback(
            np.asarray(hidden_states, np.float32),
            np.asarray(gate_kernel, np.float32),
            np.asarray(up_kernel, np.float32),
            np.asarray(down_kernel, np.float32),
            gs.astype(np.int64),
        )
    out, _ = run_sharded(
        hidden_states, gate_kernel, up_kernel, down_kernel, group_sizes
    )
    return out
